# revision 1
# baseline (speedup 1.0000x reference)
"""NLI classifier (embedding -> shared-weight LSTM x2 -> MLP) on 8 trn2 cores.

Strategy (pure data parallel):
  - 1024 sequence instances (512 from s1 + 512 from s2) sharded 128/core:
    core k owns batch rows [64k, 64k+64) of BOTH s1 (chain A) and s2 (chain B).
  - Host precompute: table2[v] = emb[v] @ w_ih.T + (b_ih+b_hh), with the g-gate
    block pre-scaled by 2 (tanh(x) = 2*sigmoid(2x) - 1 lets one Sigmoid cover
    all four gates). bf16 [50000, 1024] in DRAM, gathered on device by token.
  - Per step: PSUM gates = I64^T @ xg_t (inject) + hT^T @ w_hhT (recurrent),
    one Sigmoid over all 1024 gate columns, DVE cell update (c in fp32),
    Tanh, h in bf16, PE-transpose h into lhsT layout for the next step.
  - Two independent 64-instance chains pipeline across PE/ACT/DVE.
  - MLP head on device from the final transposed h tiles; output [3, 64] f32
    per core, host concatenates/transposes to [512, 3].
"""

import numpy as np
import ml_dtypes

import concourse.bass as bass
import concourse.bacc as bacc
import concourse.mybir as mybir
import concourse.tile as tile
from concourse.bass_utils import run_bass_kernel_spmd

BF16 = ml_dtypes.bfloat16

VOCAB = 50000
E = 128
H = 256
G = 4 * H  # 1024
B = 512
T = 256
N_CORES = 8
PB = B // N_CORES  # 64 instances per core per sequence
CH = 16            # timesteps per gather chunk

# int16 gather index encoding: idx' = v - IDX_BIAS in [-17232, 32767]
IDX_BIAS = 17232

FP32 = mybir.dt.float32
BF = mybir.dt.bfloat16
AF = mybir.ActivationFunctionType
ALU = mybir.AluOpType

GATHER_MODE = "host"  # "device" | "host"
_CACHE = {}


def _build(mode):
    nc = bacc.Bacc("TRN2", target_bir_lowering=False, debug=False,
                   num_devices=N_CORES)

    if mode == "device":
        table = nc.dram_tensor("table2", [VOCAB, G], BF, kind="ExternalInput").ap()
        idx_in = [
            nc.dram_tensor(f"idx{ch}", [128, T * 4], mybir.dt.int16,
                           kind="ExternalInput").ap()
            for ch in range(2)
        ]
    elif mode == "host":
        xg_in = [
            nc.dram_tensor(f"xg{ch}", [T, PB, G], BF, kind="ExternalInput").ap()
            for ch in range(2)
        ]
    else:  # host2
        xg_in = [
            nc.dram_tensor(f"xg{ch}", [T, PB, G], FP32, kind="ExternalInput").ap()
            for ch in range(2)
        ]
    whhT_in = nc.dram_tensor("whhT", [H, G], BF, kind="ExternalInput").ap()
    id64_in = nc.dram_tensor("id64", [PB, PB], BF, kind="ExternalInput").ap()
    id128_in = nc.dram_tensor("id128", [128, 128], BF, kind="ExternalInput").ap()
    whidT_in = nc.dram_tensor("whidT", [2 * H, H], BF, kind="ExternalInput").ap()
    bhid_in = nc.dram_tensor("bhid", [1, H], FP32, kind="ExternalInput").ap()
    woutT_in = nc.dram_tensor("woutT", [H, 3], BF, kind="ExternalInput").ap()
    bout_in = nc.dram_tensor("bout", [1, 3], FP32, kind="ExternalInput").ap()
    out_dram = nc.dram_tensor("out", [3, PB], FP32, kind="ExternalOutput").ap()

    with tile.TileContext(nc) as tc:
        with (
            tc.tile_pool(name="const", bufs=1) as cpool,
            tc.tile_pool(name="state", bufs=1) as spool,
            tc.tile_pool(name="xg", bufs=2) as xgpool,
            tc.tile_pool(name="work", bufs=2) as wpool,
            tc.tile_pool(name="gpsum", bufs=1, space="PSUM") as gpsum,
            tc.tile_pool(name="tpsum", bufs=4, space="PSUM") as tpsum,
        ):
            # ---- constants ----
            idx_sb = []
            if mode == "device":
                for ch in range(2):
                    t_ = cpool.tile([128, T * 4], mybir.dt.int16, tag=f"idx{ch}",
                                    name=f"idxs{ch}")
                    nc.sync.dma_start(out=t_[:], in_=idx_in[ch][:, :])
                    idx_sb.append(t_)
            whhT = cpool.tile([128, 2, G], BF, tag="whhT")
            nc.sync.dma_start(out=whhT[:],
                              in_=whhT_in.rearrange("(k p) g -> p k g", p=128))
            id64 = cpool.tile([PB, PB], BF, tag="id64")
            nc.sync.dma_start(out=id64[:], in_=id64_in[:, :])
            id128 = cpool.tile([128, 128], BF, tag="id128")
            nc.sync.dma_start(out=id128[:], in_=id128_in[:, :])
            whidT = cpool.tile([128, 4, H], BF, tag="whidT")
            nc.sync.dma_start(out=whidT[:],
                              in_=whidT_in.rearrange("(k p) g -> p k g", p=128))
            bhid = cpool.tile([1, H], FP32, tag="bhid")
            nc.sync.dma_start(out=bhid[:], in_=bhid_in[:, :])
            woutT = cpool.tile([128, 2, 3], BF, tag="woutT")
            nc.sync.dma_start(out=woutT[:],
                              in_=woutT_in.rearrange("(k p) g -> p k g", p=128))
            bout = cpool.tile([1, 3], FP32, tag="bout")
            nc.sync.dma_start(out=bout[:], in_=bout_in[:, :])
            ones = cpool.tile([1, PB], FP32, tag="ones")
            nc.gpsimd.memset(ones[:], 1.0)

            # ---- per-chain state ----
            c_st = [spool.tile([PB, H], FP32, tag=f"c{ch}", name=f"c{ch}") for ch in range(2)]
            hT = [spool.tile([128, 2, PB], BF, tag=f"hT{ch}", name=f"hT{ch}") for ch in range(2)]
            sig = [spool.tile([PB, G], BF, tag=f"sig{ch}", name=f"sig{ch}") for ch in range(2)]
            g2 = [spool.tile([PB, H], BF, tag=f"g2{ch}", name=f"g2{ch}") for ch in range(2)]
            u = [spool.tile([PB, H], BF, tag=f"u{ch}", name=f"uu{ch}") for ch in range(2)]
            tc_ = [spool.tile([PB, H], BF, tag=f"tc{ch}", name=f"tct{ch}") for ch in range(2)]
            h = [spool.tile([PB, H], BF, tag=f"h{ch}", name=f"hh{ch}") for ch in range(2)]

            def emit_step(ch, t, xg_t):
                first = t == 0
                ps = gpsum.tile([PB, G], FP32, tag=f"gates{ch}")
                if mode == "host2":
                    # xg lands in PSUM via DMA; recurrent matmuls accumulate
                    nc.sync.dma_start(out=ps[:, :], in_=xg_in[ch][t, :, :])
                else:
                    nc.tensor.matmul(ps[:, 0:512], lhsT=id64[:], rhs=xg_t[0:PB, 0:512],
                                     start=True, stop=False, skip_group_check=True)
                    nc.tensor.matmul(ps[:, 512:1024], lhsT=id64[:], rhs=xg_t[0:PB, 512:1024],
                                     start=True, stop=first, skip_group_check=True)
                if not first:
                    for k in range(2):
                        for nh in range(2):
                            nc.tensor.matmul(
                                ps[:, nh * 512:(nh + 1) * 512],
                                lhsT=hT[ch][:, k, :],
                                rhs=whhT[:, k, nh * 512:(nh + 1) * 512],
                                start=False, stop=(k == 1),
                                skip_group_check=True)
                nc.scalar.activation(sig[ch][:], ps[:], AF.Sigmoid)
                s = sig[ch]
                # g = 2*sigmoid(2x)-1 ; u = i*g
                nc.vector.tensor_scalar(g2[ch][:], s[:, 512:768], 2.0, -1.0,
                                        op0=ALU.mult, op1=ALU.add)
                nc.vector.tensor_tensor(u[ch][:], s[:, 0:256], g2[ch][:], op=ALU.mult)
                if first:
                    nc.vector.tensor_copy(c_st[ch][:], u[ch][:])
                else:
                    nc.vector.tensor_tensor(c_st[ch][:], s[:, 256:512], c_st[ch][:],
                                            op=ALU.mult)
                    nc.vector.tensor_tensor(c_st[ch][:], c_st[ch][:], u[ch][:],
                                            op=ALU.add)
                nc.scalar.activation(tc_[ch][:], c_st[ch][:], AF.Tanh)
                nc.vector.tensor_tensor(h[ch][:], s[:, 768:1024], tc_[ch][:],
                                        op=ALU.mult)
                for k in range(2):
                    tp = tpsum.tile([128, PB], BF, tag="tp")
                    nc.tensor.transpose(tp[:], h[ch][:, k * 128:(k + 1) * 128],
                                        id64[:])
                    nc.vector.tensor_copy(hT[ch][:, k, :], tp[:])

            n_chunks = T // CH
            for chunk in range(n_chunks):
                xg = []
                for ch in range(2):
                    if mode == "host2":
                        xg.append(None)
                        continue
                    xt = xgpool.tile([128, CH, G], BF, tag=f"xg{ch}",
                                     name=f"xgt{ch}")
                    if mode == "device":
                        for j in range(CH):
                            t = chunk * CH + j
                            nc.gpsimd.dma_gather(
                                out_ap=xt[:, j:j + 1, :],
                                in_ap=table[IDX_BIAS:, :],
                                idxs_ap=idx_sb[ch][:, t * 4:(t + 1) * 4],
                                num_idxs=PB, num_idxs_reg=PB, elem_size=G,
                            )
                    else:
                        nc.sync.dma_start(
                            out=xt[0:PB, :, :],
                            in_=xg_in[ch][chunk * CH:(chunk + 1) * CH, :, :]
                                .rearrange("c p g -> p c g"))
                    xg.append(xt)
                for j in range(CH):
                    for ch in range(2):
                        xgs = xg[ch]
                        emit_step(ch, chunk * CH + j,
                                  None if xgs is None else xgs[:, j, :])

            # ---- MLP head ----
            # catT: K-tiles [hA0, hA1, hB0, hB1], each [128, PB] bf16
            catT = [hT[0][:, 0, :], hT[0][:, 1, :], hT[1][:, 0, :], hT[1][:, 1, :]]
            hidT = wpool.tile([128, 2, PB], BF, tag="hidT")
            for m in range(2):
                hp = tpsum.tile([128, PB], FP32, tag="tp")
                for k4 in range(4):
                    nc.tensor.matmul(hp[:], lhsT=whidT[:, k4, m * 128:(m + 1) * 128],
                                     rhs=catT[k4], start=(k4 == 0), stop=False,
                                     skip_group_check=True)
                nc.tensor.matmul(hp[:], lhsT=bhid[:, m * 128:(m + 1) * 128],
                                 rhs=ones[:], start=False, stop=True,
                                 skip_group_check=True)
                nc.scalar.activation(hidT[:, m, :], hp[:], AF.Relu)
            lp = tpsum.tile([3, PB], FP32, tag="tp")
            for m in range(2):
                nc.tensor.matmul(lp[:], lhsT=woutT[:, m, :], rhs=hidT[:, m, :],
                                 start=(m == 0), stop=False, skip_group_check=True)
            nc.tensor.matmul(lp[:], lhsT=bout[:], rhs=ones[:], start=False,
                             stop=True, skip_group_check=True)
            logits = wpool.tile([3, PB], FP32, tag="logits")
            nc.vector.tensor_copy(logits[:], lp[:])
            nc.sync.dma_start(out=out_dram[:, :], in_=logits[:])

    nc.compile()
    return nc


def _wrap_idx(tok_2d):
    """tok_2d [PB, T] int -> wrapped int16 [16, T*4]: per-t 64 idx at
    (i%16, t*4 + i//16), biased by IDX_BIAS."""
    out = np.zeros((16, T * 4), np.int16)
    v = (tok_2d.astype(np.int64) - IDX_BIAS).astype(np.int16)
    for t in range(T):
        col = v[:, t]
        out[:, t * 4:(t + 1) * 4] = col.reshape(4, 16).T
    return np.tile(out, (8, 1))


LAST_RESULT = None


def kernel(s1, s2, emb, w_ih, w_hh, b_ih, b_hh, w_hid, b_hid, w_out, b_out,
           _trace=False):
    global LAST_RESULT
    s1 = np.asarray(s1)
    s2 = np.asarray(s2)
    emb = np.asarray(emb, np.float32)
    w_ih = np.asarray(w_ih, np.float32)
    w_hh = np.asarray(w_hh, np.float32)
    b_ih = np.asarray(b_ih, np.float32)
    b_hh = np.asarray(b_hh, np.float32)
    w_hid = np.asarray(w_hid, np.float32)
    b_hid = np.asarray(b_hid, np.float32)
    w_out = np.asarray(w_out, np.float32)
    b_out = np.asarray(b_out, np.float32)

    # host precompute: projected+biased gate table, g block scaled by 2
    scale = np.ones((G, 1), np.float32)
    scale[512:768] = 2.0
    Wg = w_ih * scale
    bias = ((b_ih + b_hh) * scale[:, 0])
    table2_f32 = (emb @ Wg.T + bias).astype(np.float32)  # [V, G]
    table2 = table2_f32.astype(BF16)
    whhT = (w_hh.T * scale[:, 0]).astype(BF16)      # [H, G]

    mode = GATHER_MODE
    if mode not in _CACHE:
        _CACHE[mode] = _build(mode)
    nc = _CACHE[mode]

    id64 = np.eye(PB, dtype=BF16)
    id128 = np.eye(128, dtype=BF16)
    whidT = w_hid.T.astype(BF16)                    # [512, 256]
    woutT = w_out.T.astype(BF16)                    # [256, 3]

    in_maps = []
    for k in range(N_CORES):
        sl = slice(k * PB, (k + 1) * PB)
        if mode == "device":
            gi = {
                "table2": table2,
                "idx0": _wrap_idx(s1[sl]),
                "idx1": _wrap_idx(s2[sl]),
            }
        elif mode == "host":
            gi = {
                "xg0": np.ascontiguousarray(
                    table2[s1[sl]].transpose(1, 0, 2)),
                "xg1": np.ascontiguousarray(
                    table2[s2[sl]].transpose(1, 0, 2)),
            }
        else:
            gi = {
                "xg0": np.ascontiguousarray(
                    table2_f32[s1[sl]].transpose(1, 0, 2)),
                "xg1": np.ascontiguousarray(
                    table2_f32[s2[sl]].transpose(1, 0, 2)),
            }
        in_maps.append({
            **gi,
            "whhT": whhT,
            "id64": id64,
            "id128": id128,
            "whidT": whidT,
            "bhid": b_hid.reshape(1, H).astype(np.float32),
            "woutT": woutT,
            "bout": b_out.reshape(1, 3).astype(np.float32),
        })

    res = run_bass_kernel_spmd(nc, in_maps, list(range(N_CORES)), trace=_trace)
    LAST_RESULT = res
    out = np.empty((B, 3), np.float32)
    for k in range(N_CORES):
        out[k * PB:(k + 1) * PB] = res.results[k]["out"].T
    return out



# revision 2
# speedup vs baseline: 1.1376x; 1.1376x over previous
"""NLI classifier (embedding -> shared-weight LSTM x2 -> MLP) on 8 trn2 cores.

v2 strategy (merged chains + hidden-half anti-phase pipeline):
  - 1024 sequence instances (512 s1 + 512 s2) sharded 128/core: core k owns
    s1 rows [64k,64k+64) as partitions 0-63 and s2 rows as partitions 64-127
    of ONE 128-row LSTM chain (engine costs are free-dim bound, so merging
    halves instruction count vs two 64-row chains).
  - Host precompute: table2[v] = emb[v] @ w_ih.T + (b_ih+b_hh), g-gate block
    pre-scaled by 2 (tanh(x) = 2*sigmoid(2x)-1), columns permuted so each
    hidden-half h owns one 512-wide PSUM bank laid out [i_h|f_h|g_h|o_h].
  - Per step, per half: PSUM bank = inject (id128 matmul of gathered xg)
    + hT_k0 @ whhT + hT_k1 @ whhT; one Sigmoid -> sig_h; DVE cell update
    (c fp32); PE-transpose c_h; Tanh on transposed c; hT_h = sigo_hT * tcT
    (single DVE mult, no separate h tile / copy).
  - The two halves carry the recurrence in anti-phase: hT_0(t) is ready
    early and feeds k0 matmuls of t+1 while the hi half still computes.
  - MLP head on device from hT tiles; output [3, 64] f32 per core.
"""

import numpy as np
import ml_dtypes

import concourse.bass as bass
import concourse.bacc as bacc
import concourse.mybir as mybir
import concourse.tile as tile
from concourse.bass_utils import run_bass_kernel_spmd

BF16 = ml_dtypes.bfloat16

VOCAB = 50000
E = 128
H = 256
G = 4 * H  # 1024
B = 512
T = 256
N_CORES = 8
PB = B // N_CORES  # 64 examples per core; 128 LSTM rows per core
CH = 16            # timesteps per DMA chunk

FP32 = mybir.dt.float32
BF = mybir.dt.bfloat16
AF = mybir.ActivationFunctionType
ALU = mybir.AluOpType

_CACHE = {}


def _gate_perm():
    """Column permutation: new col 512*h + 128*blk + u  <- old gate layout.

    Old rows of stacked W: i:0-255, f:256-511, g:512-767, o:768-1023.
    New: half h in {0,1} (hidden units 128h..128h+127), blocks [i,f,g,o].
    """
    perm = np.empty(G, np.int64)
    for h in range(2):
        for blk, base in enumerate((0, 256, 512, 768)):
            perm[512 * h + 128 * blk:512 * h + 128 * (blk + 1)] = \
                np.arange(base + 128 * h, base + 128 * (h + 1))
    return perm


def _build():
    nc = bacc.Bacc("TRN2", target_bir_lowering=False, debug=False,
                   num_devices=N_CORES)

    xg_in = nc.dram_tensor("xg", [T, 128, G], BF, kind="ExternalInput").ap()
    whhT_in = nc.dram_tensor("whhT", [H, G], BF, kind="ExternalInput").ap()
    id128_in = nc.dram_tensor("id128", [128, 128], BF, kind="ExternalInput").ap()
    idf32_in = nc.dram_tensor("idf32", [128, 128], FP32, kind="ExternalInput").ap()
    whidT_in = nc.dram_tensor("whidT", [2 * H, H], BF, kind="ExternalInput").ap()
    bhid_in = nc.dram_tensor("bhid", [1, H], FP32, kind="ExternalInput").ap()
    woutT_in = nc.dram_tensor("woutT", [H, 3], BF, kind="ExternalInput").ap()
    bout_in = nc.dram_tensor("bout", [1, 3], FP32, kind="ExternalOutput"
                             if False else "ExternalInput").ap()
    out_dram = nc.dram_tensor("out", [3, PB], FP32, kind="ExternalOutput").ap()

    with tile.TileContext(nc) as tc:
        with (
            tc.tile_pool(name="const", bufs=1) as cpool,
            tc.tile_pool(name="state", bufs=1) as spool,
            tc.tile_pool(name="sig", bufs=2) as sgpool,
            tc.tile_pool(name="xg", bufs=2) as xgpool,
            tc.tile_pool(name="work", bufs=2) as wpool,
            tc.tile_pool(name="glo", bufs=2, space="PSUM") as glo_pool,
            tc.tile_pool(name="ghi", bufs=2, space="PSUM") as ghi_pool,
            tc.tile_pool(name="tps", bufs=2, space="PSUM") as tps_pool,
        ):
            # ---- constants ----
            whhT = cpool.tile([128, 2, G], BF, tag="whhT")
            nc.sync.dma_start(out=whhT[:],
                              in_=whhT_in.rearrange("(k p) g -> p k g", p=128))
            id128 = cpool.tile([128, 128], BF, tag="id128")
            nc.sync.dma_start(out=id128[:], in_=id128_in[:, :])
            idf32 = cpool.tile([128, 128], FP32, tag="idf32")
            nc.sync.dma_start(out=idf32[:], in_=idf32_in[:, :])
            whidT = cpool.tile([128, 4, H], BF, tag="whidT")
            nc.sync.dma_start(out=whidT[:],
                              in_=whidT_in.rearrange("(k p) g -> p k g", p=128))
            bhid = cpool.tile([1, H], FP32, tag="bhid")
            nc.sync.dma_start(out=bhid[:], in_=bhid_in[:, :])
            woutT = cpool.tile([128, 2, 3], BF, tag="woutT")
            nc.sync.dma_start(out=woutT[:],
                              in_=woutT_in.rearrange("(k p) g -> p k g", p=128))
            bout = cpool.tile([1, 3], FP32, tag="bout")
            nc.sync.dma_start(out=bout[:], in_=bout_in[:, :])
            ones = cpool.tile([1, PB], FP32, tag="ones")
            nc.gpsimd.memset(ones[:], 1.0)

            # ---- state ----
            c_st = spool.tile([128, H], FP32, tag="c")      # cell, [rows, unit]
            hT = spool.tile([128, 2, 128], BF, tag="hT")    # [unit_in_half, k, row]
            g2 = [spool.tile([128, 128], BF, tag=f"g2{h}", name=f"g2{h}")
                  for h in range(2)]
            uu = [spool.tile([128, 128], BF, tag=f"u{h}", name=f"u{h}")
                  for h in range(2)]

            gates_cur = [None, None]   # PSUM tiles for step t
            gates_nxt = [None, None]   # pre-injected PSUM tiles for step t+1
            xg_tiles = [None, None]    # chunk ring

            def inject(t):
                """Emit inject matmuls for step t into fresh PSUM tiles."""
                ci = (t // CH) % 2
                j = t % CH
                xt = xg_tiles[ci]
                res = []
                for h in range(2):
                    pool = glo_pool if h == 0 else ghi_pool
                    ps = pool.tile([128, 512], FP32, tag=f"g{h}")
                    nc.tensor.matmul(ps[:], lhsT=id128[:],
                                     rhs=xt[:, j, 512 * h:512 * (h + 1)],
                                     start=True, stop=(t == 0),
                                     skip_group_check=True)
                    res.append(ps)
                return res

            def dma_chunk(chunk):
                ci = chunk % 2
                xt = xgpool.tile([128, CH, G], BF, tag=f"xg{ci}",
                                 name=f"xgt{ci}")
                nc.sync.dma_start(
                    out=xt[:],
                    in_=xg_in[chunk * CH:(chunk + 1) * CH, :, :]
                        .rearrange("c p g -> p c g"))
                xg_tiles[ci] = xt

            n_chunks = T // CH
            dma_chunk(0)
            for chunk in range(n_chunks):
                if chunk + 1 < n_chunks:
                    dma_chunk(chunk + 1)
                for j in range(CH):
                    t = chunk * CH + j
                    if t == 0:
                        gates_cur[0], gates_cur[1] = inject(0)
                    else:
                        gates_cur[0], gates_cur[1] = gates_nxt[0], gates_nxt[1]
                        # recurrent: k0 both halves first (need hT_0 only),
                        # then k1 both halves (need hT_1)
                        for k in range(2):
                            for h in range(2):
                                nc.tensor.matmul(
                                    gates_cur[h][:],
                                    lhsT=hT[:, k, :],
                                    rhs=whhT[:, k, 512 * h:512 * (h + 1)],
                                    start=False, stop=(k == 1),
                                    skip_group_check=True)
                    if t + 1 < T:
                        gates_nxt[0], gates_nxt[1] = inject(t + 1)

                    # per-half: sigmoid -> cell -> transpose(c) -> tanh -> hT
                    sig = [None, None]
                    for h in range(2):
                        s = sgpool.tile([128, 512], BF, tag=f"sig{h}",
                                        name=f"sig{h}")
                        nc.scalar.activation(s[:], gates_cur[h][:], AF.Sigmoid)
                        sig[h] = s
                    for h in range(2):
                        s = sig[h]
                        ch_sl = c_st[:, 128 * h:128 * (h + 1)]
                        # g2 = 2*sig(2g)-1 ; u = i*g2
                        nc.vector.tensor_scalar(g2[h][:], s[:, 256:384], 2.0,
                                                -1.0, op0=ALU.mult, op1=ALU.add)
                        nc.vector.tensor_tensor(uu[h][:], s[:, 0:128], g2[h][:],
                                                op=ALU.mult)
                        if t == 0:
                            nc.vector.tensor_copy(ch_sl, uu[h][:])
                        else:
                            nc.vector.tensor_tensor(ch_sl, s[:, 128:256], ch_sl,
                                                    op=ALU.mult)
                            nc.vector.tensor_tensor(ch_sl, ch_sl, uu[h][:],
                                                    op=ALU.add)
                    for h in range(2):
                        s = sig[h]
                        ch_sl = c_st[:, 128 * h:128 * (h + 1)]
                        oT = tps_pool.tile([128, 128], BF, tag="oT",
                                           name=f"oT{h}")
                        nc.tensor.transpose(oT[:], s[:, 384:512], id128[:])
                        cT = tps_pool.tile([128, 128], FP32, tag="cT",
                                           name=f"cT{h}")
                        nc.tensor.transpose(cT[:], ch_sl, idf32[:])
                        tcT = wpool.tile([128, 128], BF, tag=f"tcT{h}",
                                         name=f"tcT{h}")
                        nc.scalar.activation(tcT[:], cT[:], AF.Tanh)
                        nc.vector.tensor_tensor(hT[:, h, :], oT[:], tcT[:],
                                                op=ALU.mult)

            # ---- MLP head ----
            catT = [hT[:, 0, 0:PB], hT[:, 1, 0:PB],
                    hT[:, 0, PB:2 * PB], hT[:, 1, PB:2 * PB]]
            hidT = wpool.tile([128, 2, PB], BF, tag="hidT")
            for m in range(2):
                hp = tps_pool.tile([128, PB], FP32, tag="cT", name=f"hp{m}")
                for k4 in range(4):
                    nc.tensor.matmul(hp[:], lhsT=whidT[:, k4, m * 128:(m + 1) * 128],
                                     rhs=catT[k4], start=(k4 == 0), stop=False,
                                     skip_group_check=True)
                nc.tensor.matmul(hp[:], lhsT=bhid[:, m * 128:(m + 1) * 128],
                                 rhs=ones[:], start=False, stop=True,
                                 skip_group_check=True)
                nc.scalar.activation(hidT[:, m, :], hp[:], AF.Relu)
            lp = tps_pool.tile([3, PB], FP32, tag="oT", name="lp")
            for m in range(2):
                nc.tensor.matmul(lp[:], lhsT=woutT[:, m, :], rhs=hidT[:, m, :],
                                 start=(m == 0), stop=False,
                                 skip_group_check=True)
            nc.tensor.matmul(lp[:], lhsT=bout[:], rhs=ones[:], start=False,
                             stop=True, skip_group_check=True)
            logits = wpool.tile([3, PB], FP32, tag="logits")
            nc.vector.tensor_copy(logits[:], lp[:])
            nc.sync.dma_start(out=out_dram[:, :], in_=logits[:])

    nc.compile()
    return nc


LAST_RESULT = None


def kernel(s1, s2, emb, w_ih, w_hh, b_ih, b_hh, w_hid, b_hid, w_out, b_out,
           _trace=False):
    global LAST_RESULT
    s1 = np.asarray(s1)
    s2 = np.asarray(s2)
    emb = np.asarray(emb, np.float32)
    w_ih = np.asarray(w_ih, np.float32)
    w_hh = np.asarray(w_hh, np.float32)
    b_ih = np.asarray(b_ih, np.float32)
    b_hh = np.asarray(b_hh, np.float32)
    w_hid = np.asarray(w_hid, np.float32)
    b_hid = np.asarray(b_hid, np.float32)
    w_out = np.asarray(w_out, np.float32)
    b_out = np.asarray(b_out, np.float32)

    # host precompute: projected+biased gate table, g block scaled by 2,
    # columns permuted to per-half [i|f|g|o] layout
    perm = _gate_perm()
    scale = np.ones((G, 1), np.float32)
    scale[512:768] = 2.0
    Wg = w_ih * scale
    bias = ((b_ih + b_hh) * scale[:, 0])
    table2 = (emb @ Wg.T + bias)[:, perm].astype(BF16)   # [V, G]
    whhT = (w_hh.T * scale[:, 0])[:, perm].astype(BF16)  # [H, G]

    if "v2" not in _CACHE:
        _CACHE["v2"] = _build()
    nc = _CACHE["v2"]

    id128 = np.eye(128, dtype=BF16)
    idf32 = np.eye(128, dtype=np.float32)
    whidT = w_hid.T.astype(BF16)                    # [512, 256]
    woutT = w_out.T.astype(BF16)                    # [256, 3]

    in_maps = []
    for k in range(N_CORES):
        sl = slice(k * PB, (k + 1) * PB)
        toks = np.concatenate([s1[sl], s2[sl]], axis=0)      # [128, T]
        xg = np.ascontiguousarray(
            table2[toks].transpose(1, 0, 2))                 # [T, 128, G]
        in_maps.append({
            "xg": xg,
            "whhT": whhT,
            "id128": id128,
            "idf32": idf32,
            "whidT": whidT,
            "bhid": b_hid.reshape(1, H).astype(np.float32),
            "woutT": woutT,
            "bout": b_out.reshape(1, 3).astype(np.float32),
        })

    res = run_bass_kernel_spmd(nc, in_maps, list(range(N_CORES)), trace=_trace)
    LAST_RESULT = res
    out = np.empty((B, 3), np.float32)
    for k in range(N_CORES):
        out[k * PB:(k + 1) * PB] = res.results[k]["out"].T
    return out


# revision 3
# speedup vs baseline: 1.1417x; 1.0036x over previous
"""NLI classifier (embedding -> shared-weight LSTM x2 -> MLP) on 8 trn2 cores.

v2 strategy (merged chains + hidden-half anti-phase pipeline):
  - 1024 sequence instances (512 s1 + 512 s2) sharded 128/core: core k owns
    s1 rows [64k,64k+64) as partitions 0-63 and s2 rows as partitions 64-127
    of ONE 128-row LSTM chain (engine costs are free-dim bound, so merging
    halves instruction count vs two 64-row chains).
  - Host precompute: table2[v] = emb[v] @ w_ih.T + (b_ih+b_hh), g-gate block
    pre-scaled by 2 (tanh(x) = 2*sigmoid(2x)-1), columns permuted so each
    hidden-half h owns one 512-wide PSUM bank laid out [i_h|f_h|g_h|o_h].
  - Per step, per half: PSUM bank = inject (id128 matmul of gathered xg)
    + hT_k0 @ whhT + hT_k1 @ whhT; one Sigmoid -> sig_h; DVE cell update
    (c fp32); PE-transpose c_h; Tanh on transposed c; hT_h = sigo_hT * tcT
    (single DVE mult, no separate h tile / copy).
  - The two halves carry the recurrence in anti-phase: hT_0(t) is ready
    early and feeds k0 matmuls of t+1 while the hi half still computes.
  - MLP head on device from hT tiles; output [3, 64] f32 per core.
"""

import numpy as np
import ml_dtypes

import concourse.bass as bass
import concourse.bacc as bacc
import concourse.mybir as mybir
import concourse.tile as tile
from concourse.bass_utils import run_bass_kernel_spmd

BF16 = ml_dtypes.bfloat16

VOCAB = 50000
E = 128
H = 256
G = 4 * H  # 1024
B = 512
T = 256
N_CORES = 8
PB = B // N_CORES  # 64 examples per core; 128 LSTM rows per core
CH = 16            # timesteps per DMA chunk

FP32 = mybir.dt.float32
BF = mybir.dt.bfloat16
AF = mybir.ActivationFunctionType
ALU = mybir.AluOpType

OT_MODE = "pe"     # "dma" | "pe"   (sigma_o transpose path)
C_DT = "bf16"       # "bf16" | "fp32" (cell state dtype)
GU_ENG = "vector"   # "vector" | "gpsimd" (g2/u elementwise engine)
_CACHE = {}


def _gate_perm():
    """Column permutation: new col 512*h + 128*blk + u  <- old gate layout.

    Old rows of stacked W: i:0-255, f:256-511, g:512-767, o:768-1023.
    New: half h in {0,1} (hidden units 128h..128h+127), blocks [i,f,g,o].
    """
    perm = np.empty(G, np.int64)
    for h in range(2):
        for blk, base in enumerate((0, 256, 512, 768)):
            perm[512 * h + 128 * blk:512 * h + 128 * (blk + 1)] = \
                np.arange(base + 128 * h, base + 128 * (h + 1))
    return perm


def _build():
    nc = bacc.Bacc("TRN2", target_bir_lowering=False, debug=False,
                   num_devices=N_CORES)

    xg_in = nc.dram_tensor("xg", [T, 128, G], BF, kind="ExternalInput").ap()
    whhT_in = nc.dram_tensor("whhT", [H, G], BF, kind="ExternalInput").ap()
    id128_in = nc.dram_tensor("id128", [128, 128], BF, kind="ExternalInput").ap()
    idf32_in = nc.dram_tensor("idf32", [128, 128], FP32, kind="ExternalInput").ap()
    whidT_in = nc.dram_tensor("whidT", [2 * H, H], BF, kind="ExternalInput").ap()
    bhid_in = nc.dram_tensor("bhid", [1, H], FP32, kind="ExternalInput").ap()
    woutT_in = nc.dram_tensor("woutT", [H, 3], BF, kind="ExternalInput").ap()
    bout_in = nc.dram_tensor("bout", [1, 3], FP32, kind="ExternalOutput"
                             if False else "ExternalInput").ap()
    out_dram = nc.dram_tensor("out", [3, PB], FP32, kind="ExternalOutput").ap()

    with tile.TileContext(nc) as tc:
        with (
            tc.tile_pool(name="const", bufs=1) as cpool,
            tc.tile_pool(name="state", bufs=1) as spool,
            tc.tile_pool(name="sig", bufs=2) as sgpool,
            tc.tile_pool(name="xg", bufs=2) as xgpool,
            tc.tile_pool(name="work", bufs=2) as wpool,
            tc.tile_pool(name="glo", bufs=2, space="PSUM") as glo_pool,
            tc.tile_pool(name="ghi", bufs=2, space="PSUM") as ghi_pool,
            tc.tile_pool(name="tps", bufs=2, space="PSUM") as tps_pool,
        ):
            # ---- constants ----
            whhT = cpool.tile([128, 2, G], BF, tag="whhT")
            nc.sync.dma_start(out=whhT[:],
                              in_=whhT_in.rearrange("(k p) g -> p k g", p=128))
            id128 = cpool.tile([128, 128], BF, tag="id128")
            nc.sync.dma_start(out=id128[:], in_=id128_in[:, :])
            idf32 = cpool.tile([128, 128], FP32, tag="idf32")
            nc.sync.dma_start(out=idf32[:], in_=idf32_in[:, :])
            whidT = cpool.tile([128, 4, H], BF, tag="whidT")
            nc.sync.dma_start(out=whidT[:],
                              in_=whidT_in.rearrange("(k p) g -> p k g", p=128))
            bhid = cpool.tile([1, H], FP32, tag="bhid")
            nc.sync.dma_start(out=bhid[:], in_=bhid_in[:, :])
            woutT = cpool.tile([128, 2, 3], BF, tag="woutT")
            nc.sync.dma_start(out=woutT[:],
                              in_=woutT_in.rearrange("(k p) g -> p k g", p=128))
            bout = cpool.tile([1, 3], FP32, tag="bout")
            nc.sync.dma_start(out=bout[:], in_=bout_in[:, :])
            ones = cpool.tile([1, PB], FP32, tag="ones")
            nc.gpsimd.memset(ones[:], 1.0)

            # ---- state ----
            CDT = BF if C_DT == "bf16" else FP32
            c_st = spool.tile([128, H], CDT, tag="c")       # cell, [rows, unit]
            hT = spool.tile([128, 2, 128], BF, tag="hT")    # [unit_in_half, k, row]
            g2 = [spool.tile([128, 128], BF, tag=f"g2{h}", name=f"g2{h}")
                  for h in range(2)]
            uu = [spool.tile([128, 128], BF, tag=f"u{h}", name=f"u{h}")
                  for h in range(2)]

            gates_cur = [None, None]   # PSUM tiles for step t
            gates_nxt = [None, None]   # pre-injected PSUM tiles for step t+1
            xg_tiles = [None, None]    # chunk ring

            def inject(t, h):
                """Emit the inject matmul for step t, half h."""
                ci = (t // CH) % 2
                j = t % CH
                xt = xg_tiles[ci]
                pool = glo_pool if h == 0 else ghi_pool
                ps = pool.tile([128, 512], FP32, tag=f"g{h}")
                nc.tensor.matmul(ps[:], lhsT=id128[:],
                                 rhs=xt[:, j, 512 * h:512 * (h + 1)],
                                 start=True, stop=(t == 0),
                                 skip_group_check=True)
                return ps

            def dma_chunk(chunk):
                ci = chunk % 2
                xt = xgpool.tile([128, CH, G], BF, tag=f"xg{ci}",
                                 name=f"xgt{ci}")
                # chunk loads ride the ACT hwdge queue so the per-step
                # sigma_o DMA transposes on the SP queue never wait behind
                # a 4MB transfer
                eng = nc.scalar if OT_MODE == "dma" else nc.sync
                eng.dma_start(
                    out=xt[:],
                    in_=xg_in[chunk * CH:(chunk + 1) * CH, :, :]
                        .rearrange("c p g -> p c g"))
                xg_tiles[ci] = xt

            n_chunks = T // CH
            dma_chunk(0)
            for chunk in range(n_chunks):
                if chunk + 1 < n_chunks:
                    dma_chunk(chunk + 1)
                for j in range(CH):
                    t = chunk * CH + j
                    if t == 0:
                        gates_cur[0] = inject(0, 0)
                        gates_cur[1] = inject(0, 1)
                    else:
                        gates_cur[0], gates_cur[1] = gates_nxt[0], gates_nxt[1]
                        # recurrent: k0 both halves first (need hT_0 only),
                        # then k1 both halves (need hT_1)
                        for k in range(2):
                            for h in range(2):
                                nc.tensor.matmul(
                                    gates_cur[h][:],
                                    lhsT=hT[:, k, :],
                                    rhs=whhT[:, k, 512 * h:512 * (h + 1)],
                                    start=False, stop=(k == 1),
                                    skip_group_check=True)
                    # per-half: sigmoid -> cell -> transpose(c) -> tanh -> hT
                    ew = nc.gpsimd if GU_ENG == "gpsimd" else nc.vector
                    sig = [None, None]
                    oT = [None, None]
                    for h in range(2):
                        s = sgpool.tile([128, 512], BF, tag=f"sig{h}",
                                        name=f"sig{h}")
                        nc.scalar.activation(s[:], gates_cur[h][:], AF.Sigmoid)
                        sig[h] = s
                        if OT_MODE == "dma":
                            ot = wpool.tile([128, 128], BF, tag=f"oT{h}",
                                            name=f"oTs{h}")
                            nc.sync.dma_start_transpose(ot[:], s[:, 384:512])
                            oT[h] = ot
                    for h in range(2):
                        s = sig[h]
                        ch_sl = c_st[:, 128 * h:128 * (h + 1)]
                        # g2 = 2*sig(2g)-1 ; u = i*g2
                        ew.tensor_scalar(g2[h][:], s[:, 256:384], 2.0,
                                         -1.0, op0=ALU.mult, op1=ALU.add)
                        ew.tensor_tensor(uu[h][:], s[:, 0:128], g2[h][:],
                                         op=ALU.mult)
                        if t == 0:
                            nc.vector.tensor_copy(ch_sl, uu[h][:])
                        else:
                            nc.vector.tensor_tensor(ch_sl, s[:, 128:256], ch_sl,
                                                    op=ALU.mult)
                            nc.vector.tensor_tensor(ch_sl, ch_sl, uu[h][:],
                                                    op=ALU.add)
                    for h in range(2):
                        s = sig[h]
                        ch_sl = c_st[:, 128 * h:128 * (h + 1)]
                        if t + 1 < T:
                            gates_nxt[h] = inject(t + 1, h)
                        if OT_MODE == "pe":
                            ot = tps_pool.tile([128, 128], BF, tag="oT",
                                               name=f"oT{h}")
                            nc.tensor.transpose(ot[:], s[:, 384:512], id128[:])
                            oT[h] = ot
                        cT = tps_pool.tile([128, 128], CDT, tag="cT",
                                           name=f"cT{h}")
                        nc.tensor.transpose(cT[:], ch_sl,
                                            id128[:] if C_DT == "bf16"
                                            else idf32[:])
                        tcT = wpool.tile([128, 128], BF, tag=f"tcT{h}",
                                         name=f"tcT{h}")
                        nc.scalar.activation(tcT[:], cT[:], AF.Tanh)
                        nc.vector.tensor_tensor(hT[:, h, :], oT[h][:], tcT[:],
                                                op=ALU.mult)

            # ---- MLP head ----
            catT = [hT[:, 0, 0:PB], hT[:, 1, 0:PB],
                    hT[:, 0, PB:2 * PB], hT[:, 1, PB:2 * PB]]
            hidT = wpool.tile([128, 2, PB], BF, tag="hidT")
            for m in range(2):
                hp = tps_pool.tile([128, PB], FP32, tag="cT", name=f"hp{m}")
                for k4 in range(4):
                    nc.tensor.matmul(hp[:], lhsT=whidT[:, k4, m * 128:(m + 1) * 128],
                                     rhs=catT[k4], start=(k4 == 0), stop=False,
                                     skip_group_check=True)
                nc.tensor.matmul(hp[:], lhsT=bhid[:, m * 128:(m + 1) * 128],
                                 rhs=ones[:], start=False, stop=True,
                                 skip_group_check=True)
                nc.scalar.activation(hidT[:, m, :], hp[:], AF.Relu)
            lp = tps_pool.tile([3, PB], FP32, tag="oT", name="lp")
            for m in range(2):
                nc.tensor.matmul(lp[:], lhsT=woutT[:, m, :], rhs=hidT[:, m, :],
                                 start=(m == 0), stop=False,
                                 skip_group_check=True)
            nc.tensor.matmul(lp[:], lhsT=bout[:], rhs=ones[:], start=False,
                             stop=True, skip_group_check=True)
            logits = wpool.tile([3, PB], FP32, tag="logits")
            nc.vector.tensor_copy(logits[:], lp[:])
            nc.sync.dma_start(out=out_dram[:, :], in_=logits[:])

    nc.compile()
    return nc


LAST_RESULT = None


def kernel(s1, s2, emb, w_ih, w_hh, b_ih, b_hh, w_hid, b_hid, w_out, b_out,
           _trace=False):
    global LAST_RESULT
    s1 = np.asarray(s1)
    s2 = np.asarray(s2)
    emb = np.asarray(emb, np.float32)
    w_ih = np.asarray(w_ih, np.float32)
    w_hh = np.asarray(w_hh, np.float32)
    b_ih = np.asarray(b_ih, np.float32)
    b_hh = np.asarray(b_hh, np.float32)
    w_hid = np.asarray(w_hid, np.float32)
    b_hid = np.asarray(b_hid, np.float32)
    w_out = np.asarray(w_out, np.float32)
    b_out = np.asarray(b_out, np.float32)

    # host precompute: projected+biased gate table, g block scaled by 2,
    # columns permuted to per-half [i|f|g|o] layout
    perm = _gate_perm()
    scale = np.ones((G, 1), np.float32)
    scale[512:768] = 2.0
    Wg = w_ih * scale
    bias = ((b_ih + b_hh) * scale[:, 0])
    table2 = (emb @ Wg.T + bias)[:, perm].astype(BF16)   # [V, G]
    whhT = (w_hh.T * scale[:, 0])[:, perm].astype(BF16)  # [H, G]

    key = (OT_MODE, C_DT, GU_ENG)
    if key not in _CACHE:
        _CACHE[key] = _build()
    nc = _CACHE[key]

    id128 = np.eye(128, dtype=BF16)
    idf32 = np.eye(128, dtype=np.float32)
    whidT = w_hid.T.astype(BF16)                    # [512, 256]
    woutT = w_out.T.astype(BF16)                    # [256, 3]

    in_maps = []
    for k in range(N_CORES):
        sl = slice(k * PB, (k + 1) * PB)
        toks = np.concatenate([s1[sl], s2[sl]], axis=0)      # [128, T]
        xg = np.ascontiguousarray(
            table2[toks].transpose(1, 0, 2))                 # [T, 128, G]
        in_maps.append({
            "xg": xg,
            "whhT": whhT,
            "id128": id128,
            "idf32": idf32,
            "whidT": whidT,
            "bhid": b_hid.reshape(1, H).astype(np.float32),
            "woutT": woutT,
            "bout": b_out.reshape(1, 3).astype(np.float32),
        })

    res = run_bass_kernel_spmd(nc, in_maps, list(range(N_CORES)), trace=_trace)
    LAST_RESULT = res
    out = np.empty((B, 3), np.float32)
    for k in range(N_CORES):
        out[k * PB:(k + 1) * PB] = res.results[k]["out"].T
    return out
